# revision 2
# baseline (speedup 1.0000x reference)
"""Segment-normalize kernel for trn2, 8 NeuronCores, batch-parallel.

v4 "grouped-stats, three-engine normalize" design:
- Host transposes x to [BPC*F, S] f32; a core's 4 (batch, feature-half)
  units sit as a middle free dim [128, 4, S-chunk].
- Stats: ONE grouped bn_stats per short segment ([128, 4, len<=128] ->
  [128, 4, 6]); long segments (<=512) use per-unit bn_stats. Every
  segment is a single piece, so the stats combine is a fixed 9-op chain
  per chunk (no pair-adds): it folds 1/len into the even/odd counts so
  the combine produces mean and E[x^2] directly.
- Engine split: DVE owns bn_stats + reciprocal; Pool (gpsimd) owns the
  combine + A/C coefficients; ACT owns sqrt(var+eps). The normalize
  y = x*A + C (out fp16) is split per (unit, segment) across ACT
  (long segments), Pool (mid), DVE (tiny) by greedy cost balance.
- y is stored as fp16 (halves store traffic, 8x less quantization than
  bf16); host upcasts to f32.

The device program is specialized at trace time on the segment boundary
list (derived from change_points on the host); compiled NEFFs are cached
per boundary signature.
"""

import os
import numpy as np

import concourse.bass as bass
from concourse import mybir
from concourse.bass_utils import run_bass_kernel_spmd

B, S, F = 16, 8192, 256
NCORES = 8
BPC = B // NCORES           # batches per core
NU = BPC * 2                # (batch, feature-half) units per core
EPS = 1e-5
GCL = 128                   # grouped bn_stats piece limit (4*128 <= 512)
UCL = 512                   # ungrouped bn_stats piece limit

# per-instruction cost model (ns): (fix, per-elem) busy cost
_C_DVE = (61.0, 0.521)      # tensor_scalar f32->fp16
_C_ACT = (185.5, 0.833)     # activation identity scale/bias
_C_POOL = (94.5, 1.39)      # gpsimd tensor_scalar

_cache: dict = {}


def sched_emit(spacer, oplist, preheat, gap=None):
    """Emit (fn, reads, writes) ops keeping >=gap-instruction spacing
    between a writer and any later op touching the same id (real-HW SBUF
    write-drain hazard), respecting list order for conflicting ops."""
    if gap is None:
        gap = int(os.environ.get("KERNEL_GAP", "2"))
    n = len(oplist)
    deps = [set() for _ in range(n)]
    for i in range(n):
        _, ri_, wi_ = oplist[i]
        for j in range(i):
            _, rj_, wj_ = oplist[j]
            if (set(ri_) & set(wj_)) or (set(wi_) & set(rj_)) \
               or (set(wi_) & set(wj_)):
                deps[i].add(j)
    emitted = [False] * n
    last_w = dict(preheat)
    pos = 0
    out_inst = None
    remaining = n
    while remaining:
        pick = -1
        for i in range(n):
            if emitted[i]:
                continue
            if not all(emitted[j] for j in deps[i]):
                continue
            _, rds, wrs = oplist[i]
            if all(last_w.get(x, -99) <= pos - gap for x in rds + wrs):
                pick = i
                break
        if pick < 0:
            spacer()
            pos += 1
            continue
        fn, rds, wrs = oplist[pick]
        out_inst = fn()
        emitted[pick] = True
        remaining -= 1
        for wid in wrs:
            last_w[wid] = pos
        pos += 1
    return out_inst


def _plan(change_points: np.ndarray):
    ind = (np.asarray(change_points).sum(axis=0) > 0)
    ind[0] = False
    bpos = np.flatnonzero(ind)
    starts = np.concatenate([[0], bpos]).astype(np.int64)
    ends = np.concatenate([bpos, [S]]).astype(np.int64)
    segs = [(int(s), int(e - s)) for s, e in zip(starts, ends)]
    nseg = len(segs)
    assert max(ln for _, ln in segs) <= UCL, \
        "segment longer than %d not supported by this plan" % UCL

    # ---- pack whole segments into pipeline chunks --------------------------
    tgt = int(os.environ.get("KERNEL_TGT", "1280"))
    targets = [512, 896] + [tgt] * 1000
    chunks = []          # (ks, ke) segment index ranges
    ks = 0
    ci = 0
    while ks < nseg:
        t = targets[min(ci, len(targets) - 1)]
        ke = ks
        cs = 0
        while ke < nseg and (cs == 0 or cs + segs[ke][1] <= t):
            cs += segs[ke][1]
            ke += 1
        chunks.append((ks, ke))
        ks = ke
        ci += 1
    nchunk = len(chunks)
    nsegp = (nseg + 3) // 4 * 4
    csmax = max(sum(segs[k][1] for k in range(a, b)) for a, b in chunks)

    # ---- bn_stats pieces: per segment, grouped over units when len<=GCL ----
    # pieces[k] = list of (off_in_seg, cl, grouped)
    pieces = []
    for k, (s0, ln) in enumerate(segs):
        if ln <= GCL:
            pieces.append([(0, ln, True)])
        else:
            # one piece per unit (ungrouped), whole segment (<= UCL)
            pieces.append([(0, ln, False)])

    # ---- combine consts: 1/len folded into even/odd mean weights -----------
    cep = np.zeros(nsegp * 4, np.float32)
    cop = np.zeros(nsegp * 4, np.float32)
    invp = np.zeros(nsegp * 4, np.float32)
    for k, (s0, ln) in enumerate(segs):
        ce = (ln + 1) // 2
        co = ln // 2
        cep[k * 4:(k + 1) * 4] = ce / ln
        cop[k * 4:(k + 1) * 4] = co / ln
        invp[k * 4:(k + 1) * 4] = 1.0 / ln

    # ---- per-chunk engine cost bases for the normalize balance -------------
    def bn_cost(ci_):
        a, b = chunks[ci_]
        c = 0.0
        for k in range(a, b):
            ln = segs[k][1]
            if ln <= GCL:
                c += 60.0 + 4 * ln * 1.042
            else:
                c += 4 * (60.0 + ln * 1.042)
        return c

    def comb_cost(ci_):
        a, b = chunks[ci_]
        nsc = b - a
        return 11 * (94.5 + nsc * 4 * 1.39) + 3 * (94.5 + nsc * 4 * 1.39)

    # ---- normalize work assignment (greedy makespan over 3 engines) --------
    force = os.environ.get("KERNEL_FORCE_ENG", "")
    skip = set(os.environ.get("KERNEL_SKIP_ENG", "").split(",")) - {""}
    norm_by_chunk = []   # per chunk: dict eng -> list of (u, k, off_rel, l)
    for ci_, (a, b) in enumerate(chunks):
        col0 = segs[a][0]
        items = []
        for k in range(a, b):
            s0, ln = segs[k]
            for u in range(NU):
                items.append((u, k, s0 - col0, ln))
        items.sort(key=lambda it: -it[3])
        nb = bn_cost(ci_ + 1) if ci_ + 1 < nchunk else 0.0
        loads = {"dve": nb + 400.0,
                 "act": 250.0,
                 "pool": comb_cost(ci_)}
        sel = {"dve": [], "act": [], "pool": []}
        for (u, k, o, l) in items:
            if force:
                sel[force].append((u, k, o, l))
                continue
            cost = {"dve": _C_DVE[0] + _C_DVE[1] * l,
                    "act": _C_ACT[0] + _C_ACT[1] * l,
                    "pool": _C_POOL[0] + _C_POOL[1] * l}
            for e_ in skip:
                cost.pop(e_, None)
            e = min(cost, key=lambda e_: loads[e_] + cost[e_])
            loads[e] += cost[e]
            sel[e].append((u, k, o, l))
        norm_by_chunk.append(sel)

    return dict(segs=segs, nseg=nseg, nsegp=nsegp, chunks=chunks,
                pieces=pieces, csmax=csmax, cep=cep, cop=cop, invp=invp,
                norm_by_chunk=norm_by_chunk)


def _build(plan):
    f32 = mybir.dt.float32
    f16 = mybir.dt.float16
    segs = plan["segs"]
    chunks = plan["chunks"]
    pieces = plan["pieces"]
    norm_by_chunk = plan["norm_by_chunk"]
    nseg, nsegp, csmax = plan["nseg"], plan["nsegp"], plan["csmax"]
    nchunk = len(chunks)
    NW = nchunk + 2          # pipeline windows

    nc = bass.Bass()
    xt = nc.declare_dram_parameter("xt", [NU * 128, S], f32, isOutput=False)
    cepd = nc.declare_dram_parameter("cep", [nsegp * 4], f32, isOutput=False)
    copd = nc.declare_dram_parameter("cop", [nsegp * 4], f32, isOutput=False)
    invpd = nc.declare_dram_parameter("invp", [nsegp * 4], f32, isOutput=False)
    wbd = nc.declare_dram_parameter("wbx", [128, nsegp * 4], f32,
                                    isOutput=False)
    bbd = nc.declare_dram_parameter("bbx", [128, nsegp * 4], f32,
                                    isOutput=False)
    epsd = nc.declare_dram_parameter("epsv", [128, 1], f32, isOutput=False)
    yt = nc.declare_dram_parameter("yt", [NU * 128, S], f16, isOutput=True)
    NCONST = 6

    def chunk_geom(c):
        a, b = chunks[c]
        col0 = segs[a][0]
        cs = sum(segs[k][1] for k in range(a, b))
        return a, b, col0, cs

    def dram3(dram, col0, cs, dt_sz_elems):
        ap = dram[:, :]
        return bass.AP(tensor=ap.tensor, offset=ap.offset + col0,
                       ap=[[S, 128], [128 * S, NU], [1, cs]])

    from contextlib import ExitStack
    ctx = ExitStack()
    with ctx:
        xb = [ctx.enter_context(nc.sbuf_tensor("xb%d" % i, [128, NU, csmax],
                                               f32))
              for i in range(3)]
        yb = [ctx.enter_context(nc.sbuf_tensor("yb%d" % i, [128, NU, csmax],
                                               f16))
              for i in range(2)]
        s6 = ctx.enter_context(nc.sbuf_tensor([128, nsegp, NU, 6], f32))
        t1 = ctx.enter_context(nc.sbuf_tensor([128, nsegp, NU], f32))
        t2 = ctx.enter_context(nc.sbuf_tensor([128, nsegp, NU], f32))
        t3 = ctx.enter_context(nc.sbuf_tensor([128, nsegp, NU], f32))
        t4 = ctx.enter_context(nc.sbuf_tensor([128, nsegp, NU], f32))
        t5 = ctx.enter_context(nc.sbuf_tensor([128, nsegp, NU], f32))
        tmean = ctx.enter_context(nc.sbuf_tensor([128, nsegp, NU], f32))
        tex2 = ctx.enter_context(nc.sbuf_tensor([128, nsegp, NU], f32))
        tvar = ctx.enter_context(nc.sbuf_tensor([128, nsegp, NU], f32))
        tstd = ctx.enter_context(nc.sbuf_tensor([128, nsegp, NU], f32))
        trstd = ctx.enter_context(nc.sbuf_tensor([128, nsegp, NU], f32))
        At = ctx.enter_context(nc.sbuf_tensor([128, nsegp, NU], f32))
        Ct = ctx.enter_context(nc.sbuf_tensor([128, nsegp, NU], f32))
        cept = ctx.enter_context(nc.sbuf_tensor([128, nsegp, NU], f32))
        copt = ctx.enter_context(nc.sbuf_tensor([128, nsegp, NU], f32))
        invpt = ctx.enter_context(nc.sbuf_tensor([128, nsegp, NU], f32))
        wexpt = ctx.enter_context(nc.sbuf_tensor([128, nsegp, NU], f32))
        bexpt = ctx.enter_context(nc.sbuf_tensor([128, nsegp, NU], f32))
        epst = ctx.enter_context(nc.sbuf_tensor([128, 1], f32))
        dva = ctx.enter_context(nc.sbuf_tensor([128, 2], f32))
        dac = ctx.enter_context(nc.sbuf_tensor([128, 2], f32))
        dpl = ctx.enter_context(nc.sbuf_tensor([128, 2], f32))
        LDC = ctx.enter_context(nc.semaphore("LDC"))
        LDs = [ctx.enter_context(nc.semaphore("LD%d" % i)) for i in range(3)]
        BNS = ctx.enter_context(nc.semaphore("BNS"))
        CMB = ctx.enter_context(nc.semaphore("CMB"))
        SQT = ctx.enter_context(nc.semaphore("SQT"))
        RCP = ctx.enter_context(nc.semaphore("RCP"))
        COEF = ctx.enter_context(nc.semaphore("COEF"))
        ND = ctx.enter_context(nc.semaphore("ND"))
        NA = ctx.enter_context(nc.semaphore("NA"))
        NP = ctx.enter_context(nc.semaphore("NP"))
        STs = [ctx.enter_context(nc.semaphore("ST%d" % i)) for i in range(2)]
        block = ctx.enter_context(nc.Block())

        AOP = mybir.AluOpType
        AFT = mybir.ActivationFunctionType

        def stt(eng, out, in0, in1, op1):
            return eng.scalar_tensor_tensor(
                out=out, in0=in0, scalar=0.0, in1=in1,
                op0=AOP.add, op1=op1)

        def emit_norm(eng, c, items):
            """Emit normalize ops for chunk c's items on one engine."""
            insts = []
            for (u, k, o, l) in items:
                if eng == "act":
                    i = nc.scalar.activation(
                        out=yb[c % 2][:, u, o:o + l],
                        in_=xb[c % 3][:, u, o:o + l],
                        func=AFT.Identity,
                        scale=At[:, k, u:u + 1], bias=Ct[:, k, u:u + 1])
                else:
                    e = nc.vector if eng == "dve" else nc.gpsimd
                    i = e.tensor_scalar(
                        out=yb[c % 2][:, u, o:o + l],
                        in0=xb[c % 3][:, u, o:o + l],
                        scalar1=At[:, k, u:u + 1], scalar2=Ct[:, k, u:u + 1],
                        op0=AOP.mult, op1=AOP.add)
                insts.append(i)
            return insts

        # ---------------- SP: loads + stores --------------------------------
        @block.sync
        def _(sp):
            for h in (ND, NA, NP):
                sp.sem_clear(h)
            for c in range(min(3, nchunk)):
                a, b, col0, cs = chunk_geom(c)
                sp.dma_start(out=xb[c % 3][:, :, 0:cs],
                             in_=dram3(xt, col0, cs, 4)
                             ).then_inc(LDs[c % 3], 16)
            for c in range(nchunk):
                sp.wait_ge(ND, c + 1)
                sp.wait_ge(NA, c + 1)
                sp.wait_ge(NP, c + 1)
                a, b, col0, cs = chunk_geom(c)
                sp.dma_start(out=dram3(yt, col0, cs, 2),
                             in_=yb[c % 2][:, :, 0:cs]
                             ).then_inc(STs[c % 2], 16)
                if c + 3 < nchunk:
                    a2, b2, col02, cs2 = chunk_geom(c + 3)
                    sp.dma_start(out=xb[(c + 3) % 3][:, :, 0:cs2],
                                 in_=dram3(xt, col02, cs2, 4)
                                 ).then_inc(LDs[(c + 3) % 3], 16)

        # ---------------- DVE: bn_stats, recip, tiny normalize --------------
        @block.vector
        def _(ve):
            for h in (LDs[0], LDs[1], LDs[2], LDC, SQT, COEF, STs[0], STs[1]):
                ve.sem_clear(h)
            rsp = int(os.environ.get("KERNEL_RSP", "3"))
            for w in range(NW):
                c_bn, c_rc, c_nm = w, w - 1, w - 2
                if c_bn < nchunk:
                    a, b, col0, cs = chunk_geom(c_bn)
                    ve.wait_ge(LDs[c_bn % 3], 16 * (c_bn // 3 + 1))
                    if c_bn == 0:
                        ve.wait_ge(LDC, 16 * NCONST)
                    last = None
                    for k in range(a, b):
                        s0, ln = segs[k]
                        o = s0 - col0
                        for (po, cl, grouped) in pieces[k]:
                            if grouped:
                                last = nc.vector.bn_stats(
                                    out=s6[:, k, :, :],
                                    in_=xb[c_bn % 3][:, :, o + po:o + po + cl])
                            else:
                                for u in range(NU):
                                    last = nc.vector.bn_stats(
                                        out=s6[:, k, u, :],
                                        in_=xb[c_bn % 3][:, u,
                                                         o + po:o + po + cl])
                    last.then_inc(BNS, 1)
                if 0 <= c_rc < nchunk:
                    a, b, col0, cs = chunk_geom(c_rc)
                    ve.wait_ge(SQT, c_rc + 1)
                    for _ in range(rsp):
                        nc.vector.memset(dva[:, 0:2], 0.0)
                    nc.vector.reciprocal(
                        out=trstd[:, a:b, :], in_=tstd[:, a:b, :]
                    ).then_inc(RCP, 1)
                if 0 <= c_nm < nchunk:
                    ve.wait_ge(COEF, c_nm + 1)
                    if c_nm >= 2:
                        ve.wait_ge(STs[c_nm % 2], 16 * (c_nm // 2))
                    insts = emit_norm("dve", c_nm, norm_by_chunk[c_nm]["dve"])
                    if insts:
                        insts[-1].then_inc(ND, 1)
                    else:
                        nc.vector.memset(dva[:, 0:1], 0.0).then_inc(ND, 1)

        # ---------------- ACT: sqrt + long normalize ------------------------
        @block.scalar
        def _(ac):
            for h in (CMB, COEF, STs[0], STs[1], LDC):
                ac.sem_clear(h)
            ac.wait_ge(LDC, 16 * NCONST)
            adl = int(os.environ.get("KERNEL_ACT_DELAY", "1"))
            for w in range(NW):
                c_sq, c_nm = w - 1, w - 2
                if 0 <= c_sq < nchunk:
                    a, b, col0, cs = chunk_geom(c_sq)
                    ac.wait_ge(CMB, c_sq + 1)
                    for _ in range(adl):
                        nc.scalar.activation(
                            out=dac[:, 0:1], in_=dac[:, 0:1], func=AFT.Copy)
                    nc.scalar.activation(
                        out=tstd[:, a:b, :], in_=tvar[:, a:b, :],
                        func=AFT.Sqrt, bias=epst[:, 0:1], scale=1.0
                    ).then_inc(SQT, 1)
                if 0 <= c_nm < nchunk:
                    ac.wait_ge(COEF, c_nm + 1)
                    if c_nm >= 2:
                        ac.wait_ge(STs[c_nm % 2], 16 * (c_nm // 2))
                    insts = emit_norm("act", c_nm, norm_by_chunk[c_nm]["act"])
                    if insts:
                        insts[-1].then_inc(NA, 1)
                    else:
                        nc.scalar.activation(
                            out=dac[:, 0:1], in_=dac[:, 0:1],
                            func=AFT.Copy).then_inc(NA, 1)

        # ---------------- Pool: consts, combine, coefs, mid normalize -------
        @block.gpsimd
        def _(g):
            for h in (BNS, RCP, STs[0], STs[1]):
                g.sem_clear(h)

            def bcast(dram, n):
                ap = dram[:]
                return bass.AP(tensor=ap.tensor, offset=ap.offset,
                               ap=[[0, 128], [1, n]])
            g.dma_start(out=cept[:, :, :], in_=bcast(cepd, nsegp * 4)
                        ).then_inc(LDC, 16)
            g.dma_start(out=copt[:, :, :], in_=bcast(copd, nsegp * 4)
                        ).then_inc(LDC, 16)
            g.dma_start(out=invpt[:, :, :], in_=bcast(invpd, nsegp * 4)
                        ).then_inc(LDC, 16)
            g.dma_start(out=wexpt[:, :, :], in_=wbd[:, :]).then_inc(LDC, 16)
            g.dma_start(out=bexpt[:, :, :], in_=bbd[:, :]).then_inc(LDC, 16)
            g.dma_start(out=epst[:, :], in_=epsd[:, :]).then_inc(LDC, 16)

            def spacer():
                nc.gpsimd.memset(dpl[:, 0:2], 0.0)

            for w in range(NW):
                c_cb, c_cf, c_nm = w, w - 1, w - 2
                if c_cb < nchunk:
                    a, b, col0, cs = chunk_geom(c_cb)
                    g.wait_ge(BNS, c_cb + 1)
                    if c_cb == 0:
                        g.wait_ge(LDC, 16 * NCONST)
                    m_e = s6[:, a:b, :, 1]
                    M2e = s6[:, a:b, :, 2]
                    m_o = s6[:, a:b, :, 4]
                    M2o = s6[:, a:b, :, 5]
                    sl = slice(a, b)
                    P = nc.gpsimd

                    def f(out, in0, in1, op1):
                        return lambda: stt(P, out, in0, in1, op1)
                    ops = [
                        (f(t1[:, sl, :], m_e, cept[:, sl, :], AOP.mult),
                         ["s6"], ["t1"]),
                        (f(t2[:, sl, :], m_o, copt[:, sl, :], AOP.mult),
                         ["s6"], ["t2"]),
                        (f(t3[:, sl, :], M2e, M2o, AOP.add), ["s6"], ["t3"]),
                        (f(t4[:, sl, :], m_e, t1[:, sl, :], AOP.mult),
                         ["s6", "t1"], ["t4"]),
                        (f(t5[:, sl, :], m_o, t2[:, sl, :], AOP.mult),
                         ["s6", "t2"], ["t5"]),
                        (f(tmean[:, sl, :], t1[:, sl, :], t2[:, sl, :],
                           AOP.add), ["t1", "t2"], ["mean"]),
                        (f(t3[:, sl, :], t3[:, sl, :], invpt[:, sl, :],
                           AOP.mult), ["t3"], ["t3"]),
                        (f(t4[:, sl, :], t3[:, sl, :], t4[:, sl, :], AOP.add),
                         ["t3", "t4"], ["t4"]),
                        (f(tex2[:, sl, :], t4[:, sl, :], t5[:, sl, :],
                           AOP.add), ["t4", "t5"], ["ex2"]),
                        (f(t1[:, sl, :], tmean[:, sl, :], tmean[:, sl, :],
                           AOP.mult), ["mean"], ["t1"]),
                        (f(tvar[:, sl, :], tex2[:, sl, :], t1[:, sl, :],
                           AOP.subtract), ["ex2", "t1"], ["var"]),
                    ]
                    sched_emit(spacer, ops, {"s6": 0})
                    spacer()
                    nc.gpsimd.memset(dpl[:, 0:2], 0.0).then_inc(CMB, 1)
                if 0 <= c_cf < nchunk:
                    a, b, col0, cs = chunk_geom(c_cf)
                    g.wait_ge(RCP, c_cf + 1)
                    sl = slice(a, b)
                    P = nc.gpsimd

                    def f(out, in0, in1, op1):
                        return lambda: stt(P, out, in0, in1, op1)
                    ops = [
                        (f(At[:, sl, :], trstd[:, sl, :], wexpt[:, sl, :],
                           AOP.mult), ["rstd"], ["A"]),
                        (f(t2[:, sl, :], tmean[:, sl, :], At[:, sl, :],
                           AOP.mult), ["A"], ["t2c"]),
                        (f(Ct[:, sl, :], bexpt[:, sl, :], t2[:, sl, :],
                           AOP.subtract), ["t2c"], ["C"]),
                    ]
                    sched_emit(spacer, ops, {"rstd": 0})
                    spacer()
                    nc.gpsimd.memset(dpl[:, 0:2], 0.0).then_inc(COEF, 1)
                if 0 <= c_nm < nchunk:
                    if c_nm >= 2:
                        g.wait_ge(STs[c_nm % 2], 16 * (c_nm // 2))
                    insts = emit_norm("pool", c_nm,
                                      norm_by_chunk[c_nm]["pool"])
                    if insts:
                        insts[-1].then_inc(NP, 1)
                    else:
                        nc.gpsimd.memset(dpl[:, 0:1], 0.0).then_inc(NP, 1)

    return nc


def kernel(x, affine_weight, affine_bias, change_points):
    x = np.asarray(x, dtype=np.float32)
    w = np.asarray(affine_weight, dtype=np.float32)
    bb = np.asarray(affine_bias, dtype=np.float32)
    cp = np.asarray(change_points)

    plan = _plan(cp)
    sig = tuple(s for s, _ in plan["segs"])
    if sig not in _cache:
        _cache[sig] = _build(plan)
    nc = _cache[sig]

    nsegp = plan["nsegp"]
    wbx = np.zeros((128, nsegp * 4), np.float32)
    bbx = np.zeros((128, nsegp * 4), np.float32)
    for u in range(NU):
        fh = u % 2
        wbx[:, u::4] = w[fh * 128:(fh + 1) * 128][:, None]
        bbx[:, u::4] = bb[fh * 128:(fh + 1) * 128][:, None]
    epsv = np.full((128, 1), EPS, np.float32)

    in_maps = []
    for i in range(NCORES):
        xt = np.ascontiguousarray(
            x[i * BPC:(i + 1) * BPC].transpose(0, 2, 1)).reshape(NU * 128, S)
        in_maps.append({"xt": xt, "cep": plan["cep"], "cop": plan["cop"],
                        "invp": plan["invp"], "wbx": wbx, "bbx": bbx,
                        "epsv": epsv})

    res = run_bass_kernel_spmd(nc, in_maps, core_ids=list(range(NCORES)),
                               trace=False)

    y = np.empty((B, S, F), np.float32)
    for i in range(NCORES):
        yt = np.asarray(res.results[i]["yt"]).astype(np.float32)
        y[i * BPC:(i + 1) * BPC] = yt.reshape(BPC, F, S).transpose(0, 2, 1)
    return y


# revision 3
# speedup vs baseline: 1.7567x; 1.7567x over previous
"""Segment-normalize kernel for trn2, 8 NeuronCores, batch-parallel.

v4 "grouped-stats, three-engine normalize" design:
- Host transposes x to [BPC*F, S] f32; a core's 4 (batch, feature-half)
  units sit as a middle free dim [128, 4, S-chunk].
- Stats: ONE grouped bn_stats per short segment ([128, 4, len<=128] ->
  [128, 4, 6]); long segments (<=512) use per-unit bn_stats. Every
  segment is a single piece, so the stats combine is a fixed 11-op chain
  per chunk (no pair-adds): it folds 1/len into the even/odd counts so
  the combine produces mean and E[x^2] directly.
- Engine split: DVE owns bn_stats + reciprocal + A/C coefficients; Pool
  (gpsimd) owns the combine; ACT owns sqrt(var+eps). The normalize
  y = x*A + C (out fp16) is split per (unit, segment) across ACT
  (long segments), Pool (mid), DVE (tiny) by greedy cost balance.
- 4 input / 3 output SBUF buffers keep the DMA queue streaming; loads
  and stores use one 3D DMA per chunk (all 4 units).
- y is stored as fp16 (halves store traffic, 8x less quantization than
  bf16); host upcasts to f32.

The device program is specialized at trace time on the segment boundary
list (derived from change_points on the host); compiled NEFFs are cached
per boundary signature.
"""

import os
import numpy as np

import concourse.bass as bass
from concourse import mybir
from concourse.bass_utils import run_bass_kernel_spmd

B, S, F = 16, 8192, 256
NCORES = 8
BPC = B // NCORES           # batches per core
NU = BPC * 2                # (batch, feature-half) units per core
EPS = 1e-5
GCL = 128                   # grouped bn_stats piece limit (4*128 <= 512)
UCL = 512                   # ungrouped bn_stats piece limit
NBUF = 4                    # input chunk buffers
YBUF = 3                    # output chunk buffers

# per-instruction cost model (ns): (fix, per-elem) busy cost
_C_DVE = (61.0, 0.521)      # tensor_scalar f32->fp16
_C_ACT = (185.5, 0.833)     # activation identity scale/bias
_C_POOL = (94.5, 1.39)      # gpsimd tensor_scalar

_cache: dict = {}


def sched_emit(spacer, oplist, preheat, gap=None):
    """Emit (fn, reads, writes) ops keeping >=gap-instruction spacing
    between a writer and any later op touching the same id (real-HW SBUF
    write-drain hazard), respecting list order for conflicting ops."""
    if gap is None:
        gap = int(os.environ.get("KERNEL_GAP", "2"))
    n = len(oplist)
    deps = [set() for _ in range(n)]
    for i in range(n):
        _, ri_, wi_ = oplist[i]
        for j in range(i):
            _, rj_, wj_ = oplist[j]
            if (set(ri_) & set(wj_)) or (set(wi_) & set(rj_)) \
               or (set(wi_) & set(wj_)):
                deps[i].add(j)
    emitted = [False] * n
    last_w = dict(preheat)
    pos = 0
    out_inst = None
    remaining = n
    while remaining:
        pick = -1
        for i in range(n):
            if emitted[i]:
                continue
            if not all(emitted[j] for j in deps[i]):
                continue
            _, rds, wrs = oplist[i]
            if all(last_w.get(x, -99) <= pos - gap for x in rds + wrs):
                pick = i
                break
        if pick < 0:
            spacer()
            pos += 1
            continue
        fn, rds, wrs = oplist[pick]
        out_inst = fn()
        emitted[pick] = True
        remaining -= 1
        for wid in wrs:
            last_w[wid] = pos
        pos += 1
    return out_inst


def _plan(change_points: np.ndarray):
    ind = (np.asarray(change_points).sum(axis=0) > 0)
    ind[0] = False
    bpos = np.flatnonzero(ind)
    starts = np.concatenate([[0], bpos]).astype(np.int64)
    ends = np.concatenate([bpos, [S]]).astype(np.int64)
    segs = [(int(s), int(e - s)) for s, e in zip(starts, ends)]
    nseg = len(segs)
    assert max(ln for _, ln in segs) <= UCL, \
        "segment longer than %d not supported by this plan" % UCL

    # ---- pack whole segments into pipeline chunks --------------------------
    tgt = int(os.environ.get("KERNEL_TGT", "1280"))
    targets = [512, 896] + [tgt] * 1000
    chunks = []          # (ks, ke) segment index ranges
    ks = 0
    ci = 0
    while ks < nseg:
        t = targets[min(ci, len(targets) - 1)]
        ke = ks
        cs = 0
        while ke < nseg and (cs == 0 or cs + segs[ke][1] <= t):
            cs += segs[ke][1]
            ke += 1
        chunks.append((ks, ke))
        ks = ke
        ci += 1
    nchunk = len(chunks)
    nsegp = (nseg + 3) // 4 * 4
    csmax = max(sum(segs[k][1] for k in range(a, b)) for a, b in chunks)

    # ---- combine consts: 1/len folded into even/odd mean weights -----------
    cep = np.zeros(nsegp * 4, np.float32)
    cop = np.zeros(nsegp * 4, np.float32)
    invp = np.zeros(nsegp * 4, np.float32)
    for k, (s0, ln) in enumerate(segs):
        ce = (ln + 1) // 2
        co = ln // 2
        cep[k * 4:(k + 1) * 4] = ce / ln
        cop[k * 4:(k + 1) * 4] = co / ln
        invp[k * 4:(k + 1) * 4] = 1.0 / ln

    # ---- per-chunk engine cost bases for the normalize balance -------------
    def bn_cost(ci_):
        a, b = chunks[ci_]
        c = 0.0
        for k in range(a, b):
            ln = segs[k][1]
            if ln <= GCL:
                c += 60.0 + 4 * ln * 1.042
            else:
                c += 4 * (60.0 + ln * 1.042)
        return c

    def comb_cost(ci_):
        a, b = chunks[ci_]
        nsc = b - a
        return 13 * (94.5 + nsc * 4 * 1.39)

    # ---- normalize work assignment (greedy makespan over 3 engines) --------
    force = os.environ.get("KERNEL_FORCE_ENG", "")
    skip = set(os.environ.get("KERNEL_SKIP_ENG", "").split(",")) - {""}
    norm_by_chunk = []   # per chunk: dict eng -> list of (u, k, off_rel, l)
    for ci_, (a, b) in enumerate(chunks):
        col0 = segs[a][0]
        items = []
        for k in range(a, b):
            s0, ln = segs[k]
            for u in range(NU):
                items.append((u, k, s0 - col0, ln))
        items.sort(key=lambda it: -it[3])
        nb = bn_cost(ci_ + 1) if ci_ + 1 < nchunk else 0.0
        loads = {"dve": nb + 600.0,
                 "act": 450.0,
                 "pool": comb_cost(ci_)}
        sel = {"dve": [], "act": [], "pool": []}
        for (u, k, o, l) in items:
            if force:
                sel[force].append((u, k, o, l))
                continue
            cost = {"dve": _C_DVE[0] + _C_DVE[1] * l,
                    "act": _C_ACT[0] + _C_ACT[1] * l,
                    "pool": _C_POOL[0] + _C_POOL[1] * l}
            for e_ in skip:
                cost.pop(e_, None)
            e = min(cost, key=lambda e_: loads[e_] + cost[e_])
            loads[e] += cost[e]
            sel[e].append((u, k, o, l))
        norm_by_chunk.append(sel)

    return dict(segs=segs, nseg=nseg, nsegp=nsegp, chunks=chunks,
                csmax=csmax, cep=cep, cop=cop, invp=invp,
                norm_by_chunk=norm_by_chunk)


def _build(plan):
    f32 = mybir.dt.float32
    f16 = mybir.dt.float16
    segs = plan["segs"]
    chunks = plan["chunks"]
    norm_by_chunk = plan["norm_by_chunk"]
    nseg, nsegp, csmax = plan["nseg"], plan["nsegp"], plan["csmax"]
    nchunk = len(chunks)
    NW = nchunk + 2          # pipeline windows

    nc = bass.Bass()
    xt = nc.declare_dram_parameter("xt", [NU * 128, S], f32, isOutput=False)
    cepd = nc.declare_dram_parameter("cep", [nsegp * 4], f32, isOutput=False)
    copd = nc.declare_dram_parameter("cop", [nsegp * 4], f32, isOutput=False)
    invpd = nc.declare_dram_parameter("invp", [nsegp * 4], f32, isOutput=False)
    wbd = nc.declare_dram_parameter("wbx", [128, nsegp * 4], f32,
                                    isOutput=False)
    bbd = nc.declare_dram_parameter("bbx", [128, nsegp * 4], f32,
                                    isOutput=False)
    epsd = nc.declare_dram_parameter("epsv", [128, 1], f32, isOutput=False)
    yt = nc.declare_dram_parameter("yt", [NU * 128, S], f16, isOutput=True)
    NCONST = 6

    def chunk_geom(c):
        a, b = chunks[c]
        col0 = segs[a][0]
        cs = sum(segs[k][1] for k in range(a, b))
        return a, b, col0, cs

    def dram3(dram, col0, cs):
        ap = dram[:, :]
        return bass.AP(tensor=ap.tensor, offset=ap.offset + col0,
                       ap=[[S, 128], [128 * S, NU], [1, cs]])

    from contextlib import ExitStack
    ctx = ExitStack()
    with ctx:
        xb = [ctx.enter_context(nc.sbuf_tensor("xb%d" % i, [128, NU, csmax],
                                               f32))
              for i in range(NBUF)]
        yb = [ctx.enter_context(nc.sbuf_tensor("yb%d" % i, [128, NU, csmax],
                                               f16))
              for i in range(YBUF)]
        s6 = ctx.enter_context(nc.sbuf_tensor([128, nsegp, NU, 6], f32))
        t1 = ctx.enter_context(nc.sbuf_tensor([128, nsegp, NU], f32))
        t2 = ctx.enter_context(nc.sbuf_tensor([128, nsegp, NU], f32))
        t3 = ctx.enter_context(nc.sbuf_tensor([128, nsegp, NU], f32))
        t4 = ctx.enter_context(nc.sbuf_tensor([128, nsegp, NU], f32))
        t5 = ctx.enter_context(nc.sbuf_tensor([128, nsegp, NU], f32))
        t6 = ctx.enter_context(nc.sbuf_tensor([128, nsegp, NU], f32))
        tmean = ctx.enter_context(nc.sbuf_tensor([128, nsegp, NU], f32))
        tex2 = ctx.enter_context(nc.sbuf_tensor([128, nsegp, NU], f32))
        tvar = ctx.enter_context(nc.sbuf_tensor([128, nsegp, NU], f32))
        tstd = ctx.enter_context(nc.sbuf_tensor([128, nsegp, NU], f32))
        trstd = ctx.enter_context(nc.sbuf_tensor([128, nsegp, NU], f32))
        At = ctx.enter_context(nc.sbuf_tensor([128, nsegp, NU], f32))
        Ct = ctx.enter_context(nc.sbuf_tensor([128, nsegp, NU], f32))
        cept = ctx.enter_context(nc.sbuf_tensor([128, nsegp, NU], f32))
        copt = ctx.enter_context(nc.sbuf_tensor([128, nsegp, NU], f32))
        invpt = ctx.enter_context(nc.sbuf_tensor([128, nsegp, NU], f32))
        wexpt = ctx.enter_context(nc.sbuf_tensor([128, nsegp, NU], f32))
        bexpt = ctx.enter_context(nc.sbuf_tensor([128, nsegp, NU], f32))
        epst = ctx.enter_context(nc.sbuf_tensor([128, 1], f32))
        dva = ctx.enter_context(nc.sbuf_tensor([128, 2], f32))
        dac = ctx.enter_context(nc.sbuf_tensor([128, 2], f32))
        dpl = ctx.enter_context(nc.sbuf_tensor([128, 2], f32))
        LDC = ctx.enter_context(nc.semaphore("LDC"))
        LD = ctx.enter_context(nc.semaphore("LD"))
        BNS = ctx.enter_context(nc.semaphore("BNS"))
        CMB = ctx.enter_context(nc.semaphore("CMB"))
        SQT = ctx.enter_context(nc.semaphore("SQT"))
        COEF = ctx.enter_context(nc.semaphore("COEF"))
        ND = ctx.enter_context(nc.semaphore("ND"))
        NA = ctx.enter_context(nc.semaphore("NA"))
        NP = ctx.enter_context(nc.semaphore("NP"))
        ST = ctx.enter_context(nc.semaphore("ST"))
        block = ctx.enter_context(nc.Block())

        AOP = mybir.AluOpType
        AFT = mybir.ActivationFunctionType

        def stt(eng, out, in0, in1, op1):
            return eng.scalar_tensor_tensor(
                out=out, in0=in0, scalar=0.0, in1=in1,
                op0=AOP.add, op1=op1)

        def emit_norm(eng, c, items):
            """Emit normalize ops for chunk c's items on one engine."""
            insts = []
            for (u, k, o, l) in items:
                if eng == "act":
                    i = nc.scalar.activation(
                        out=yb[c % YBUF][:, u, o:o + l],
                        in_=xb[c % NBUF][:, u, o:o + l],
                        func=AFT.Identity,
                        scale=At[:, k, u:u + 1], bias=Ct[:, k, u:u + 1])
                else:
                    e = nc.vector if eng == "dve" else nc.gpsimd
                    i = e.tensor_scalar(
                        out=yb[c % YBUF][:, u, o:o + l],
                        in0=xb[c % NBUF][:, u, o:o + l],
                        scalar1=At[:, k, u:u + 1], scalar2=Ct[:, k, u:u + 1],
                        op0=AOP.mult, op1=AOP.add)
                insts.append(i)
            return insts

        # ---------------- SP: loads + stores --------------------------------
        @block.sync
        def _(sp):
            for h in (ND, NA, NP):
                sp.sem_clear(h)
            for c in range(min(NBUF, nchunk)):
                a, b, col0, cs = chunk_geom(c)
                sp.dma_start(out=xb[c % NBUF][:, :, 0:cs],
                             in_=dram3(xt, col0, cs)).then_inc(LD, 16)
            for c in range(nchunk):
                sp.wait_ge(ND, c + 1)
                sp.wait_ge(NA, c + 1)
                sp.wait_ge(NP, c + 1)
                a, b, col0, cs = chunk_geom(c)
                sp.dma_start(out=dram3(yt, col0, cs),
                             in_=yb[c % YBUF][:, :, 0:cs]).then_inc(ST, 16)
                if c + NBUF < nchunk:
                    a2, b2, col02, cs2 = chunk_geom(c + NBUF)
                    sp.dma_start(out=xb[(c + NBUF) % NBUF][:, :, 0:cs2],
                                 in_=dram3(xt, col02, cs2)).then_inc(LD, 16)

        # ---------------- DVE: bn_stats, recip+coefs, tiny normalize --------
        @block.vector
        def _(ve):
            for h in (LD, LDC, SQT, COEF, ST):
                ve.sem_clear(h)
            rsp = int(os.environ.get("KERNEL_RSP", "2"))
            for w in range(NW):
                c_bn, c_rc, c_nm = w, w - 1, w - 2
                if c_bn < nchunk:
                    a, b, col0, cs = chunk_geom(c_bn)
                    ve.wait_ge(LD, 16 * (c_bn + 1))
                    if c_bn == 0:
                        ve.wait_ge(LDC, 16 * NCONST)
                    last = None
                    for k in range(a, b):
                        s0, ln = segs[k]
                        o = s0 - col0
                        if ln <= GCL:
                            last = nc.vector.bn_stats(
                                out=s6[:, k, :, :],
                                in_=xb[c_bn % NBUF][:, :, o:o + ln])
                        else:
                            for u in range(NU):
                                last = nc.vector.bn_stats(
                                    out=s6[:, k, u, :],
                                    in_=xb[c_bn % NBUF][:, u, o:o + ln])
                    last.then_inc(BNS, 1)
                if 0 <= c_rc < nchunk:
                    a, b, col0, cs = chunk_geom(c_rc)
                    ve.wait_ge(SQT, c_rc + 1)
                    sl = slice(a, b)
                    for _ in range(rsp):
                        nc.vector.memset(dva[:, 0:2], 0.0)
                    nc.vector.reciprocal(
                        out=trstd[:, sl, :], in_=tstd[:, sl, :])
                    nc.vector.memset(dva[:, 0:2], 0.0)
                    nc.vector.memset(dva[:, 0:2], 0.0)
                    stt(nc.vector, At[:, sl, :], trstd[:, sl, :],
                        wexpt[:, sl, :], AOP.mult)
                    nc.vector.memset(dva[:, 0:2], 0.0)
                    nc.vector.memset(dva[:, 0:2], 0.0)
                    stt(nc.vector, t6[:, sl, :], tmean[:, sl, :],
                        At[:, sl, :], AOP.mult)
                    nc.vector.memset(dva[:, 0:2], 0.0)
                    nc.vector.memset(dva[:, 0:2], 0.0)
                    stt(nc.vector, Ct[:, sl, :], bexpt[:, sl, :],
                        t6[:, sl, :], AOP.subtract).then_inc(COEF, 1)
                if 0 <= c_nm < nchunk:
                    ve.wait_ge(COEF, c_nm + 1)
                    if c_nm >= YBUF - 1:
                        ve.wait_ge(ST, 16 * (c_nm - YBUF + 2))
                    insts = emit_norm("dve", c_nm, norm_by_chunk[c_nm]["dve"])
                    if insts:
                        insts[-1].then_inc(ND, 1)
                    else:
                        nc.vector.memset(dva[:, 0:1], 0.0).then_inc(ND, 1)

        # ---------------- ACT: sqrt + long normalize ------------------------
        @block.scalar
        def _(ac):
            for h in (CMB, COEF, ST, LDC):
                ac.sem_clear(h)
            ac.wait_ge(LDC, 16 * NCONST)
            adl = int(os.environ.get("KERNEL_ACT_DELAY", "1"))
            for w in range(NW):
                c_sq, c_nm = w - 1, w - 2
                if 0 <= c_sq < nchunk:
                    a, b, col0, cs = chunk_geom(c_sq)
                    ac.wait_ge(CMB, c_sq + 1)
                    for _ in range(adl):
                        nc.scalar.activation(
                            out=dac[:, 0:1], in_=dac[:, 0:1], func=AFT.Copy)
                    nc.scalar.activation(
                        out=tstd[:, a:b, :], in_=tvar[:, a:b, :],
                        func=AFT.Sqrt, bias=epst[:, 0:1], scale=1.0
                    ).then_inc(SQT, 1)
                if 0 <= c_nm < nchunk:
                    ac.wait_ge(COEF, c_nm + 1)
                    if c_nm >= YBUF - 1:
                        ac.wait_ge(ST, 16 * (c_nm - YBUF + 2))
                    insts = emit_norm("act", c_nm, norm_by_chunk[c_nm]["act"])
                    if insts:
                        insts[-1].then_inc(NA, 1)
                    else:
                        nc.scalar.activation(
                            out=dac[:, 0:1], in_=dac[:, 0:1],
                            func=AFT.Copy).then_inc(NA, 1)

        # ---------------- Pool: consts, mid normalize, combine --------------
        @block.gpsimd
        def _(g):
            for h in (BNS, COEF, ST):
                g.sem_clear(h)

            def bcast(dram, n):
                ap = dram[:]
                return bass.AP(tensor=ap.tensor, offset=ap.offset,
                               ap=[[0, 128], [1, n]])
            g.dma_start(out=cept[:, :, :], in_=bcast(cepd, nsegp * 4)
                        ).then_inc(LDC, 16)
            g.dma_start(out=copt[:, :, :], in_=bcast(copd, nsegp * 4)
                        ).then_inc(LDC, 16)
            g.dma_start(out=invpt[:, :, :], in_=bcast(invpd, nsegp * 4)
                        ).then_inc(LDC, 16)
            g.dma_start(out=wexpt[:, :, :], in_=wbd[:, :]).then_inc(LDC, 16)
            g.dma_start(out=bexpt[:, :, :], in_=bbd[:, :]).then_inc(LDC, 16)
            g.dma_start(out=epst[:, :], in_=epsd[:, :]).then_inc(LDC, 16)

            def spacer():
                nc.gpsimd.memset(dpl[:, 0:2], 0.0)

            for w in range(NW):
                c_cb, c_nm = w, w - 2
                if 0 <= c_nm < nchunk:
                    g.wait_ge(COEF, c_nm + 1)
                    if c_nm >= YBUF - 1:
                        g.wait_ge(ST, 16 * (c_nm - YBUF + 2))
                    insts = emit_norm("pool", c_nm,
                                      norm_by_chunk[c_nm]["pool"])
                    if insts:
                        insts[-1].then_inc(NP, 1)
                    else:
                        nc.gpsimd.memset(dpl[:, 0:1], 0.0).then_inc(NP, 1)
                if c_cb < nchunk:
                    a, b, col0, cs = chunk_geom(c_cb)
                    g.wait_ge(BNS, c_cb + 1)
                    if c_cb == 0:
                        g.wait_ge(LDC, 16 * NCONST)
                    m_e = s6[:, a:b, :, 1]
                    M2e = s6[:, a:b, :, 2]
                    m_o = s6[:, a:b, :, 4]
                    M2o = s6[:, a:b, :, 5]
                    sl = slice(a, b)
                    P = nc.gpsimd

                    def f(out, in0, in1, op1):
                        return lambda: stt(P, out, in0, in1, op1)
                    ops = [
                        (f(t1[:, sl, :], m_e, cept[:, sl, :], AOP.mult),
                         ["s6"], ["t1"]),
                        (f(t2[:, sl, :], m_o, copt[:, sl, :], AOP.mult),
                         ["s6"], ["t2"]),
                        (f(t3[:, sl, :], M2e, M2o, AOP.add), ["s6"], ["t3"]),
                        (f(t4[:, sl, :], m_e, t1[:, sl, :], AOP.mult),
                         ["s6", "t1"], ["t4"]),
                        (f(t5[:, sl, :], m_o, t2[:, sl, :], AOP.mult),
                         ["s6", "t2"], ["t5"]),
                        (f(tmean[:, sl, :], t1[:, sl, :], t2[:, sl, :],
                           AOP.add), ["t1", "t2"], ["mean"]),
                        (f(t3[:, sl, :], t3[:, sl, :], invpt[:, sl, :],
                           AOP.mult), ["t3"], ["t3"]),
                        (f(t4[:, sl, :], t3[:, sl, :], t4[:, sl, :], AOP.add),
                         ["t3", "t4"], ["t4"]),
                        (f(tex2[:, sl, :], t4[:, sl, :], t5[:, sl, :],
                           AOP.add), ["t4", "t5"], ["ex2"]),
                        (f(t1[:, sl, :], tmean[:, sl, :], tmean[:, sl, :],
                           AOP.mult), ["mean"], ["t1"]),
                        (f(tvar[:, sl, :], tex2[:, sl, :], t1[:, sl, :],
                           AOP.subtract), ["ex2", "t1"], ["var"]),
                    ]
                    last = sched_emit(spacer, ops, {"s6": 0})
                    last.then_inc(CMB, 1)

    return nc


def kernel(x, affine_weight, affine_bias, change_points):
    x = np.asarray(x, dtype=np.float32)
    w = np.asarray(affine_weight, dtype=np.float32)
    bb = np.asarray(affine_bias, dtype=np.float32)
    cp = np.asarray(change_points)

    plan = _plan(cp)
    sig = tuple(s for s, _ in plan["segs"])
    if sig not in _cache:
        _cache[sig] = _build(plan)
    nc = _cache[sig]

    nsegp = plan["nsegp"]
    wbx = np.zeros((128, nsegp * 4), np.float32)
    bbx = np.zeros((128, nsegp * 4), np.float32)
    for u in range(NU):
        fh = u % 2
        wbx[:, u::4] = w[fh * 128:(fh + 1) * 128][:, None]
        bbx[:, u::4] = bb[fh * 128:(fh + 1) * 128][:, None]
    epsv = np.full((128, 1), EPS, np.float32)

    in_maps = []
    for i in range(NCORES):
        xt = np.ascontiguousarray(
            x[i * BPC:(i + 1) * BPC].transpose(0, 2, 1)).reshape(NU * 128, S)
        in_maps.append({"xt": xt, "cep": plan["cep"], "cop": plan["cop"],
                        "invp": plan["invp"], "wbx": wbx, "bbx": bbx,
                        "epsv": epsv})

    res = run_bass_kernel_spmd(nc, in_maps, core_ids=list(range(NCORES)),
                               trace=False)

    y = np.empty((B, S, F), np.float32)
    for i in range(NCORES):
        yt = np.asarray(res.results[i]["yt"]).astype(np.float32)
        y[i * BPC:(i + 1) * BPC] = yt.reshape(BPC, F, S).transpose(0, 2, 1)
    return y


# revision 6
# speedup vs baseline: 2.0258x; 1.1532x over previous
"""Segment-normalize kernel for trn2, 8 NeuronCores, batch-parallel.

v4 "grouped-stats, three-engine normalize" design:
- Host transposes x to [BPC*F, S] f32; a core's 4 (batch, feature-half)
  units sit as a middle free dim [128, 4, S-chunk].
- Stats: ONE grouped bn_stats per short segment ([128, 4, len<=128] ->
  [128, 4, 6]); long segments (<=512) use per-unit bn_stats. Every
  segment is a single piece, so the stats combine is a fixed 11-op chain
  per chunk (no pair-adds): it folds 1/len into the even/odd counts so
  the combine produces mean and E[x^2] directly.
- Engine split: DVE owns bn_stats + reciprocal + A/C coefficients; Pool
  (gpsimd) owns the combine; ACT owns sqrt(var+eps). The normalize
  y = x*A + C (out fp16) is split per (unit, segment) across ACT
  (long segments), Pool (mid), DVE (tiny) by greedy cost balance.
- 4 input / 3 output SBUF buffers keep the DMA queue streaming; loads
  and stores use one 3D DMA per chunk (all 4 units).
- y is stored as fp16 (halves store traffic, 8x less quantization than
  bf16); host upcasts to f32.

The device program is specialized at trace time on the segment boundary
list (derived from change_points on the host); compiled NEFFs are cached
per boundary signature.
"""

import os
import numpy as np

import concourse.bass as bass
from concourse import mybir
from concourse.bass_utils import run_bass_kernel_spmd

B, S, F = 16, 8192, 256
NCORES = 8
BPC = B // NCORES           # batches per core
NU = BPC * 2                # (batch, feature-half) units per core
EPS = 1e-5
GCL = 128                   # grouped bn_stats piece limit (4*128 <= 512)
UCL = 512                   # ungrouped bn_stats piece limit
NBUF = 4                    # input chunk buffers
YBUF = 3                    # output chunk buffers

# per-instruction cost model (ns): (fix, per-elem) busy cost
_C_DVE = (61.0, 0.521)      # tensor_scalar f32->fp16
_C_ACT = (185.5, 0.833)     # activation identity scale/bias
_C_POOL = (94.5, 1.39)      # gpsimd tensor_scalar

_cache: dict = {}


def sched_emit(spacer, oplist, preheat, gap=None):
    """Emit (fn, reads, writes) ops keeping >=gap-instruction spacing
    between a writer and any later op touching the same id (real-HW SBUF
    write-drain hazard), respecting list order for conflicting ops."""
    if gap is None:
        gap = int(os.environ.get("KERNEL_GAP", "2"))
    n = len(oplist)
    deps = [set() for _ in range(n)]
    for i in range(n):
        _, ri_, wi_ = oplist[i]
        for j in range(i):
            _, rj_, wj_ = oplist[j]
            if (set(ri_) & set(wj_)) or (set(wi_) & set(rj_)) \
               or (set(wi_) & set(wj_)):
                deps[i].add(j)
    emitted = [False] * n
    last_w = dict(preheat)
    pos = 0
    out_inst = None
    remaining = n
    while remaining:
        pick = -1
        for i in range(n):
            if emitted[i]:
                continue
            if not all(emitted[j] for j in deps[i]):
                continue
            _, rds, wrs = oplist[i]
            if all(last_w.get(x, -99) <= pos - gap for x in rds + wrs):
                pick = i
                break
        if pick < 0:
            spacer()
            pos += 1
            continue
        fn, rds, wrs = oplist[pick]
        out_inst = fn()
        emitted[pick] = True
        remaining -= 1
        for wid in wrs:
            last_w[wid] = pos
        pos += 1
    return out_inst


def _plan(change_points: np.ndarray):
    ind = (np.asarray(change_points).sum(axis=0) > 0)
    ind[0] = False
    bpos = np.flatnonzero(ind)
    starts = np.concatenate([[0], bpos]).astype(np.int64)
    ends = np.concatenate([bpos, [S]]).astype(np.int64)
    segs = [(int(s), int(e - s)) for s, e in zip(starts, ends)]
    nseg = len(segs)
    assert max(ln for _, ln in segs) <= UCL, \
        "segment longer than %d not supported by this plan" % UCL

    # ---- pack whole segments into pipeline chunks --------------------------
    tgt = int(os.environ.get("KERNEL_TGT", "1280"))
    targets = [512, 896] + [tgt] * 1000
    chunks = []          # (ks, ke) segment index ranges
    ks = 0
    ci = 0
    while ks < nseg:
        t = targets[min(ci, len(targets) - 1)]
        ke = ks
        cs = 0
        while ke < nseg and (cs == 0 or cs + segs[ke][1] <= t):
            cs += segs[ke][1]
            ke += 1
        chunks.append((ks, ke))
        ks = ke
        ci += 1
    nchunk = len(chunks)
    nsegp = (nseg + 3) // 4 * 4
    csmax = max(sum(segs[k][1] for k in range(a, b)) for a, b in chunks)

    # ---- combine consts: 1/len folded into even/odd mean weights -----------
    cep = np.zeros(nsegp * 4, np.float32)
    cop = np.zeros(nsegp * 4, np.float32)
    invp = np.zeros(nsegp * 4, np.float32)
    for k, (s0, ln) in enumerate(segs):
        ce = (ln + 1) // 2
        co = ln // 2
        cep[k * 4:(k + 1) * 4] = ce / ln
        cop[k * 4:(k + 1) * 4] = co / ln
        invp[k * 4:(k + 1) * 4] = 1.0 / ln

    # ---- per-chunk engine cost bases for the normalize balance -------------
    def bn_cost(ci_):
        a, b = chunks[ci_]
        c = 0.0
        for k in range(a, b):
            ln = segs[k][1]
            if ln <= GCL:
                c += 60.0 + 4 * ln * 1.042
            else:
                c += 4 * (60.0 + ln * 1.042)
        return c

    def comb_cost(ci_):
        a, b = chunks[ci_]
        nsc = b - a
        return 13 * (94.5 + nsc * 4 * 1.39)

    # ---- normalize work assignment (greedy makespan over 3 engines) --------
    force = os.environ.get("KERNEL_FORCE_ENG", "")
    skip = set(os.environ.get("KERNEL_SKIP_ENG", "").split(",")) - {""}
    norm_by_chunk = []   # per chunk: dict eng -> list of (u, k, off_rel, l)
    for ci_, (a, b) in enumerate(chunks):
        col0 = segs[a][0]
        items = []
        for k in range(a, b):
            s0, ln = segs[k]
            for u in range(NU):
                items.append((u, k, s0 - col0, ln))
        items.sort(key=lambda it: -it[3])
        nb = bn_cost(ci_ + 1) if ci_ + 1 < nchunk else 0.0
        loads = {"dve": nb + 600.0,
                 "act": 450.0,
                 "pool": comb_cost(ci_)}
        sel = {"dve": [], "act": [], "pool": []}
        for (u, k, o, l) in items:
            if force:
                sel[force].append((u, k, o, l))
                continue
            cost = {"dve": _C_DVE[0] + _C_DVE[1] * l,
                    "act": _C_ACT[0] + _C_ACT[1] * l,
                    "pool": _C_POOL[0] + _C_POOL[1] * l}
            for e_ in skip:
                cost.pop(e_, None)
            e = min(cost, key=lambda e_: loads[e_] + cost[e_])
            loads[e] += cost[e]
            sel[e].append((u, k, o, l))
        norm_by_chunk.append(sel)

    return dict(segs=segs, nseg=nseg, nsegp=nsegp, chunks=chunks,
                csmax=csmax, cep=cep, cop=cop, invp=invp,
                norm_by_chunk=norm_by_chunk)


def _build(plan):
    f32 = mybir.dt.float32
    f16 = mybir.dt.float16
    segs = plan["segs"]
    chunks = plan["chunks"]
    norm_by_chunk = plan["norm_by_chunk"]
    nseg, nsegp, csmax = plan["nseg"], plan["nsegp"], plan["csmax"]
    nchunk = len(chunks)
    NW = nchunk + 2          # pipeline windows

    nc = bass.Bass()
    xt = nc.declare_dram_parameter("xt", [NU * 128, S], f32, isOutput=False)
    cepd = nc.declare_dram_parameter("cep", [nsegp * 4], f32, isOutput=False)
    copd = nc.declare_dram_parameter("cop", [nsegp * 4], f32, isOutput=False)
    invpd = nc.declare_dram_parameter("invp", [nsegp * 4], f32, isOutput=False)
    wbd = nc.declare_dram_parameter("wbx", [128, nsegp * 4], f32,
                                    isOutput=False)
    bbd = nc.declare_dram_parameter("bbx", [128, nsegp * 4], f32,
                                    isOutput=False)
    epsd = nc.declare_dram_parameter("epsv", [128, 1], f32, isOutput=False)
    yt = nc.declare_dram_parameter("yt", [NU * 128, S], f16, isOutput=True)
    NCONST = 6

    def chunk_geom(c):
        a, b = chunks[c]
        col0 = segs[a][0]
        cs = sum(segs[k][1] for k in range(a, b))
        return a, b, col0, cs

    def dram3(dram, col0, cs):
        ap = dram[:, :]
        return bass.AP(tensor=ap.tensor, offset=ap.offset + col0,
                       ap=[[S, 128], [128 * S, NU], [1, cs]])

    from contextlib import ExitStack
    ctx = ExitStack()
    with ctx:
        xb = [ctx.enter_context(nc.sbuf_tensor("xb%d" % i, [128, NU, csmax],
                                               f32))
              for i in range(NBUF)]
        yb = [ctx.enter_context(nc.sbuf_tensor("yb%d" % i, [128, NU, csmax],
                                               f16))
              for i in range(YBUF)]
        s6 = ctx.enter_context(nc.sbuf_tensor([128, nsegp, NU, 6], f32))
        t1 = ctx.enter_context(nc.sbuf_tensor([128, nsegp, NU], f32))
        t2 = ctx.enter_context(nc.sbuf_tensor([128, nsegp, NU], f32))
        t3 = ctx.enter_context(nc.sbuf_tensor([128, nsegp, NU], f32))
        t4 = ctx.enter_context(nc.sbuf_tensor([128, nsegp, NU], f32))
        t5 = ctx.enter_context(nc.sbuf_tensor([128, nsegp, NU], f32))
        t6 = ctx.enter_context(nc.sbuf_tensor([128, nsegp, NU], f32))
        tmean = ctx.enter_context(nc.sbuf_tensor([128, nsegp, NU], f32))
        tex2 = ctx.enter_context(nc.sbuf_tensor([128, nsegp, NU], f32))
        tvar = ctx.enter_context(nc.sbuf_tensor([128, nsegp, NU], f32))
        tstd = ctx.enter_context(nc.sbuf_tensor([128, nsegp, NU], f32))
        trstd = ctx.enter_context(nc.sbuf_tensor([128, nsegp, NU], f32))
        At = ctx.enter_context(nc.sbuf_tensor([128, nsegp, NU], f32))
        Ct = ctx.enter_context(nc.sbuf_tensor([128, nsegp, NU], f32))
        cept = ctx.enter_context(nc.sbuf_tensor([128, nsegp, NU], f32))
        copt = ctx.enter_context(nc.sbuf_tensor([128, nsegp, NU], f32))
        invpt = ctx.enter_context(nc.sbuf_tensor([128, nsegp, NU], f32))
        wexpt = ctx.enter_context(nc.sbuf_tensor([128, nsegp, NU], f32))
        bexpt = ctx.enter_context(nc.sbuf_tensor([128, nsegp, NU], f32))
        epst = ctx.enter_context(nc.sbuf_tensor([128, 1], f32))
        dva = ctx.enter_context(nc.sbuf_tensor([128, 2], f32))
        dac = ctx.enter_context(nc.sbuf_tensor([128, 2], f32))
        dpl = ctx.enter_context(nc.sbuf_tensor([128, 2], f32))
        LDC = ctx.enter_context(nc.semaphore("LDC"))
        LD = ctx.enter_context(nc.semaphore("LD"))
        BNS = ctx.enter_context(nc.semaphore("BNS"))
        CMB = ctx.enter_context(nc.semaphore("CMB"))
        SQT = ctx.enter_context(nc.semaphore("SQT"))
        COEF = ctx.enter_context(nc.semaphore("COEF"))
        ND = ctx.enter_context(nc.semaphore("ND"))
        NA = ctx.enter_context(nc.semaphore("NA"))
        NP = ctx.enter_context(nc.semaphore("NP"))
        ST = ctx.enter_context(nc.semaphore("ST"))
        block = ctx.enter_context(nc.Block())

        AOP = mybir.AluOpType
        AFT = mybir.ActivationFunctionType

        def stt(eng, out, in0, in1, op1):
            return eng.scalar_tensor_tensor(
                out=out, in0=in0, scalar=0.0, in1=in1,
                op0=AOP.add, op1=op1)

        def emit_norm(eng, c, items):
            """Emit normalize ops for chunk c's items on one engine."""
            insts = []
            for (u, k, o, l) in items:
                if eng == "act":
                    i = nc.scalar.activation(
                        out=yb[c % YBUF][:, u, o:o + l],
                        in_=xb[c % NBUF][:, u, o:o + l],
                        func=AFT.Identity,
                        scale=At[:, k, u:u + 1], bias=Ct[:, k, u:u + 1])
                else:
                    e = nc.vector if eng == "dve" else nc.gpsimd
                    i = e.tensor_scalar(
                        out=yb[c % YBUF][:, u, o:o + l],
                        in0=xb[c % NBUF][:, u, o:o + l],
                        scalar1=At[:, k, u:u + 1], scalar2=Ct[:, k, u:u + 1],
                        op0=AOP.mult, op1=AOP.add)
                insts.append(i)
            return insts

        # ---------------- SP: loads + stores --------------------------------
        @block.sync
        def _(sp):
            for h in (ND, NA, NP):
                sp.sem_clear(h)

            def bcast(dram, n):
                ap = dram[:]
                return bass.AP(tensor=ap.tensor, offset=ap.offset,
                               ap=[[0, 128], [1, n]])
            sp.dma_start(out=cept[:, :, :], in_=bcast(cepd, nsegp * 4)
                         ).then_inc(LDC, 16)
            sp.dma_start(out=copt[:, :, :], in_=bcast(copd, nsegp * 4)
                         ).then_inc(LDC, 16)
            sp.dma_start(out=invpt[:, :, :], in_=bcast(invpd, nsegp * 4)
                         ).then_inc(LDC, 16)
            sp.dma_start(out=wexpt[:, :, :], in_=wbd[:, :]).then_inc(LDC, 16)
            sp.dma_start(out=bexpt[:, :, :], in_=bbd[:, :]).then_inc(LDC, 16)
            sp.dma_start(out=epst[:, :], in_=epsd[:, :]).then_inc(LDC, 16)
            for c in range(min(NBUF, nchunk)):
                a, b, col0, cs = chunk_geom(c)
                sp.dma_start(out=xb[c % NBUF][:, :, 0:cs],
                             in_=dram3(xt, col0, cs)).then_inc(LD, 16)
            for c in range(nchunk):
                sp.wait_ge(ND, c + 1)
                sp.wait_ge(NA, c + 1)
                sp.wait_ge(NP, c + 1)
                a, b, col0, cs = chunk_geom(c)
                sp.dma_start(out=dram3(yt, col0, cs),
                             in_=yb[c % YBUF][:, :, 0:cs]).then_inc(ST, 16)
                if c + NBUF < nchunk:
                    a2, b2, col02, cs2 = chunk_geom(c + NBUF)
                    sp.dma_start(out=xb[(c + NBUF) % NBUF][:, :, 0:cs2],
                                 in_=dram3(xt, col02, cs2)).then_inc(LD, 16)

        # ---------------- DVE: bn_stats, recip+coefs, tiny normalize --------
        @block.vector
        def _(ve):
            for h in (LD, LDC, SQT, COEF, ST):
                ve.sem_clear(h)
            rsp = int(os.environ.get("KERNEL_RSP", "2"))
            for w in range(NW):
                c_bn, c_rc, c_nm = w, w - 1, w - 2
                if c_bn < nchunk:
                    a, b, col0, cs = chunk_geom(c_bn)
                    ve.wait_ge(LD, 16 * (c_bn + 1))
                    last = None
                    for k in range(a, b):
                        s0, ln = segs[k]
                        o = s0 - col0
                        if ln <= GCL:
                            last = nc.vector.bn_stats(
                                out=s6[:, k, :, :],
                                in_=xb[c_bn % NBUF][:, :, o:o + ln])
                        else:
                            for u in range(NU):
                                last = nc.vector.bn_stats(
                                    out=s6[:, k, u, :],
                                    in_=xb[c_bn % NBUF][:, u, o:o + ln])
                    last.then_inc(BNS, 1)
                # normalize items of chunk c_nm double as write-drain spacers
                # for the recip/coef chain of chunk c_rc.
                nitems = norm_by_chunk[c_nm]["dve"] \
                    if 0 <= c_nm < nchunk else []
                ni = 0

                def filler(n, c_nm=c_nm, nitems=nitems):
                    nonlocal ni
                    out = []
                    take = nitems[ni:ni + n]
                    if take:
                        out = emit_norm("dve", c_nm, take)
                        ni += len(take)
                    for _ in range(n - len(take)):
                        out.append(nc.vector.memset(dva[:, 0:2], 0.0))
                    return out

                if 0 <= c_nm < nchunk:
                    ve.wait_ge(COEF, c_nm + 1)
                    if c_nm >= YBUF - 1:
                        ve.wait_ge(ST, 16 * (c_nm - YBUF + 2))
                if 0 <= c_rc < nchunk:
                    if c_rc == 0:
                        ve.wait_ge(LDC, 16 * NCONST)
                    a, b, col0, cs = chunk_geom(c_rc)
                    ve.wait_ge(SQT, c_rc + 1)
                    sl = slice(a, b)
                    filler(rsp)
                    nc.vector.reciprocal(
                        out=trstd[:, sl, :], in_=tstd[:, sl, :])
                    filler(2)
                    stt(nc.vector, At[:, sl, :], trstd[:, sl, :],
                        wexpt[:, sl, :], AOP.mult)
                    filler(2)
                    stt(nc.vector, t6[:, sl, :], tmean[:, sl, :],
                        At[:, sl, :], AOP.mult)
                    filler(2)
                    stt(nc.vector, Ct[:, sl, :], bexpt[:, sl, :],
                        t6[:, sl, :], AOP.subtract).then_inc(COEF, 1)
                if 0 <= c_nm < nchunk:
                    insts = filler(max(0, len(nitems) - ni))
                    if insts:
                        insts[-1].then_inc(ND, 1)
                    else:
                        nc.vector.memset(dva[:, 0:1], 0.0).then_inc(ND, 1)

        # ---------------- ACT: sqrt + long normalize ------------------------
        @block.scalar
        def _(ac):
            for h in (CMB, COEF, ST, LDC):
                ac.sem_clear(h)
            ac.wait_ge(LDC, 16 * NCONST)
            adl = int(os.environ.get("KERNEL_ACT_DELAY", "1"))
            for w in range(NW):
                c_sq, c_nm = w - 1, w - 2
                if 0 <= c_sq < nchunk:
                    a, b, col0, cs = chunk_geom(c_sq)
                    ac.wait_ge(CMB, c_sq + 1)
                    for _ in range(adl):
                        nc.scalar.activation(
                            out=dac[:, 0:1], in_=dac[:, 0:1], func=AFT.Copy)
                    nc.scalar.activation(
                        out=tstd[:, a:b, :], in_=tvar[:, a:b, :],
                        func=AFT.Sqrt, bias=epst[:, 0:1], scale=1.0
                    ).then_inc(SQT, 1)
                if 0 <= c_nm < nchunk:
                    ac.wait_ge(COEF, c_nm + 1)
                    if c_nm >= YBUF - 1:
                        ac.wait_ge(ST, 16 * (c_nm - YBUF + 2))
                    insts = emit_norm("act", c_nm, norm_by_chunk[c_nm]["act"])
                    if insts:
                        insts[-1].then_inc(NA, 1)
                    else:
                        nc.scalar.activation(
                            out=dac[:, 0:1], in_=dac[:, 0:1],
                            func=AFT.Copy).then_inc(NA, 1)

        # ---------------- Pool: consts, mid normalize, combine --------------
        @block.gpsimd
        def _(g):
            for h in (BNS, COEF, ST, LDC):
                g.sem_clear(h)

            def spacer():
                nc.gpsimd.memset(dpl[:, 0:2], 0.0)

            for w in range(NW):
                c_cb, c_nm = w, w - 2
                if 0 <= c_nm < nchunk:
                    g.wait_ge(COEF, c_nm + 1)
                    if c_nm >= YBUF - 1:
                        g.wait_ge(ST, 16 * (c_nm - YBUF + 2))
                    insts = emit_norm("pool", c_nm,
                                      norm_by_chunk[c_nm]["pool"])
                    if insts:
                        insts[-1].then_inc(NP, 1)
                    else:
                        nc.gpsimd.memset(dpl[:, 0:1], 0.0).then_inc(NP, 1)
                if c_cb < nchunk:
                    a, b, col0, cs = chunk_geom(c_cb)
                    g.wait_ge(BNS, c_cb + 1)
                    if c_cb == 0:
                        g.wait_ge(LDC, 16 * NCONST)
                    m_e = s6[:, a:b, :, 1]
                    M2e = s6[:, a:b, :, 2]
                    m_o = s6[:, a:b, :, 4]
                    M2o = s6[:, a:b, :, 5]
                    sl = slice(a, b)
                    P = nc.gpsimd

                    def f(out, in0, in1, op1):
                        return lambda: stt(P, out, in0, in1, op1)
                    ops = [
                        (f(t1[:, sl, :], m_e, cept[:, sl, :], AOP.mult),
                         ["s6"], ["t1"]),
                        (f(t2[:, sl, :], m_o, copt[:, sl, :], AOP.mult),
                         ["s6"], ["t2"]),
                        (f(t3[:, sl, :], M2e, M2o, AOP.add), ["s6"], ["t3"]),
                        (f(t4[:, sl, :], m_e, t1[:, sl, :], AOP.mult),
                         ["s6", "t1"], ["t4"]),
                        (f(t5[:, sl, :], m_o, t2[:, sl, :], AOP.mult),
                         ["s6", "t2"], ["t5"]),
                        (f(tmean[:, sl, :], t1[:, sl, :], t2[:, sl, :],
                           AOP.add), ["t1", "t2"], ["mean"]),
                        (f(t3[:, sl, :], t3[:, sl, :], invpt[:, sl, :],
                           AOP.mult), ["t3"], ["t3"]),
                        (f(t4[:, sl, :], t3[:, sl, :], t4[:, sl, :], AOP.add),
                         ["t3", "t4"], ["t4"]),
                        (f(tex2[:, sl, :], t4[:, sl, :], t5[:, sl, :],
                           AOP.add), ["t4", "t5"], ["ex2"]),
                        (f(t1[:, sl, :], tmean[:, sl, :], tmean[:, sl, :],
                           AOP.mult), ["mean"], ["t1"]),
                        (f(tvar[:, sl, :], tex2[:, sl, :], t1[:, sl, :],
                           AOP.subtract), ["ex2", "t1"], ["var"]),
                    ]
                    last = sched_emit(spacer, ops, {"s6": 0})
                    last.then_inc(CMB, 1)

    return nc


def kernel(x, affine_weight, affine_bias, change_points):
    x = np.asarray(x, dtype=np.float32)
    w = np.asarray(affine_weight, dtype=np.float32)
    bb = np.asarray(affine_bias, dtype=np.float32)
    cp = np.asarray(change_points)

    plan = _plan(cp)
    sig = tuple(s for s, _ in plan["segs"])
    if sig not in _cache:
        _cache[sig] = _build(plan)
    nc = _cache[sig]

    nsegp = plan["nsegp"]
    wbx = np.zeros((128, nsegp * 4), np.float32)
    bbx = np.zeros((128, nsegp * 4), np.float32)
    for u in range(NU):
        fh = u % 2
        wbx[:, u::4] = w[fh * 128:(fh + 1) * 128][:, None]
        bbx[:, u::4] = bb[fh * 128:(fh + 1) * 128][:, None]
    epsv = np.full((128, 1), EPS, np.float32)

    in_maps = []
    for i in range(NCORES):
        xt = np.ascontiguousarray(
            x[i * BPC:(i + 1) * BPC].transpose(0, 2, 1)).reshape(NU * 128, S)
        in_maps.append({"xt": xt, "cep": plan["cep"], "cop": plan["cop"],
                        "invp": plan["invp"], "wbx": wbx, "bbx": bbx,
                        "epsv": epsv})

    res = run_bass_kernel_spmd(nc, in_maps, core_ids=list(range(NCORES)),
                               trace=False)

    y = np.empty((B, S, F), np.float32)
    for i in range(NCORES):
        yt = np.asarray(res.results[i]["yt"]).astype(np.float32)
        y[i * BPC:(i + 1) * BPC] = yt.reshape(BPC, F, S).transpose(0, 2, 1)
    return y


# revision 8
# speedup vs baseline: 2.3879x; 1.1787x over previous
"""Segment-normalize kernel for trn2, 8 NeuronCores, batch-parallel.

v5 "feature-pair interleave" design:
- Host stages each core's slice as [2*128 rows, 2*S cols] f32: row
  (u*128+p) interleaves features p and p+128 of batch u along columns
  (x[u,c,p] at col 2c, x[u,c,p+128] at col 2c+1).
- bn_stats' even/odd split then yields EXACT per-feature stats: one
  bn_stats per (batch-unit, segment) covers two features per partition
  ([128, 2*len<=512] -> 6-tuple = (len, mean_f0, len*var_f0, len,
  mean_f1, len*var_f1)). Mean needs no recombine; var = M2 * (1/len)
  is a single gpsimd op per chunk. Segments longer than 256 use two
  interleaved pieces recombined with immediate coefficients (rare).
- Chain per chunk: DVE bn_stats -> Pool var -> ACT sqrt(var+eps) ->
  DVE recip + coefficients (trivial affine: A=rstd, C=-mean*rstd) ->
  normalize y = x*A + C (out fp16) split per (unit, segment, parity)
  across ACT (long), Pool (mid), DVE (tiny) by greedy cost balance,
  using stride-2 column APs.
- Deep buffering (6 in / 5 out) keeps the DMA engines streaming; one
  3D DMA per chunk each way.

The device program is specialized at trace time on the segment boundary
list (derived from change_points on the host); compiled NEFFs are cached
per boundary signature.
"""

import os
import numpy as np

import concourse.bass as bass
from concourse import mybir
from concourse.bass_utils import run_bass_kernel_spmd

B, S, F = 16, 8192, 256
NCORES = 8
BPC = B // NCORES           # batches per core
NU = BPC                    # batch units per core (feature pairs interleaved)
NL = 4                      # stat lanes per (seg): (unit, parity)
EPS = 1e-5
ICL = 256                   # interleaved bn_stats piece limit (2*256 <= 512)
S2 = 2 * S
NBUF = int(os.environ.get("KERNEL_NBUF", "6"))   # input chunk buffers
YBUF = int(os.environ.get("KERNEL_YBUF", "5"))   # output chunk buffers

# per-instruction cost model (ns): (fix, per-elem) busy cost
_C_DVE = (61.0, 0.521)      # tensor_scalar f32->fp16
_C_ACT = (185.5, 0.833)     # activation identity scale/bias
_C_POOL = (94.5, 1.39)      # gpsimd tensor_scalar

_cache: dict = {}


def _plan(change_points: np.ndarray, triv: bool = True):
    ind = (np.asarray(change_points).sum(axis=0) > 0)
    ind[0] = False
    bpos = np.flatnonzero(ind)
    starts = np.concatenate([[0], bpos]).astype(np.int64)
    ends = np.concatenate([bpos, [S]]).astype(np.int64)
    segs = [(int(s), int(e - s)) for s, e in zip(starts, ends)]
    nseg = len(segs)
    assert max(ln for _, ln in segs) <= 2 * ICL, "segment longer than 512"

    # ---- pack whole segments into pipeline chunks --------------------------
    tgt = int(os.environ.get("KERNEL_TGT", "768"))
    ramp = [256, 512] + ([768] if tgt > 768 else [])
    tail = [t for t in (512, 320, 192) if t < tgt]

    def pack(seg_range, targets):
        ks, ke_end = seg_range
        out = []
        ci = 0
        while ks < ke_end:
            t = targets[min(ci, len(targets) - 1)]
            ke = ks
            cs = 0
            while ke < ke_end and (cs == 0 or cs + segs[ke][1] <= t):
                cs += segs[ke][1]
                ke += 1
            out.append((ks, ke))
            ks = ke
            ci += 1
        return out

    chunks = pack((0, nseg), ramp + [tgt] * 1000)
    if len(chunks) > len(ramp) + 1:
        last = chunks.pop()
        chunks += pack(last, tail + [192] * 1000)
    nchunk = len(chunks)

    # ---- long segments (> ICL) get a second piece in an extra s6 column ----
    xcol = {}                # seg k -> extra s6 column for its 2nd piece
    for k, (s0, ln) in enumerate(segs):
        if ln > ICL:
            xcol[k] = nseg + len(xcol)
    nsegx = nseg + len(xcol)
    nsegp = (nsegx + 3) // 4 * 4
    csmax = max(sum(segs[k][1] for k in range(a, b)) for a, b in chunks)

    # ---- var const: 1/len per segment --------------------------------------
    invp = np.zeros(nsegp, np.float16)
    for k, (s0, ln) in enumerate(segs):
        invp[k] = 1.0 / ln

    # ---- per-chunk engine cost bases for the normalize balance -------------
    def bn_cost(ci_):
        a, b = chunks[ci_]
        c = 0.0
        for k in range(a, b):
            ln = segs[k][1]
            n_p = 1 if ln <= ICL else 2
            c += NU * n_p * 60.0 + NU * 2 * ln * 1.042
        return c

    # ---- normalize work assignment (greedy makespan over 3 engines) --------
    force = os.environ.get("KERNEL_FORCE_ENG", "")
    skip = set(os.environ.get("KERNEL_SKIP_ENG", "").split(",")) - {""}
    norm_by_chunk = []   # per chunk: dict eng -> list of (u, k, off_rel, l, p)
    for ci_, (a, b) in enumerate(chunks):
        col0 = segs[a][0]
        items = []
        for k in range(a, b):
            s0, ln = segs[k]
            for u in range(NU):
                for par in range(2):
                    items.append((u, k, s0 - col0, ln, par))
        items.sort(key=lambda it: -it[3])
        nb = bn_cost(ci_ + 1) if ci_ + 1 < nchunk else 0.0
        loads = {"dve": nb + 600.0,
                 "act": 450.0,
                 "pool": 400.0}
        sel = {"dve": [], "act": [], "pool": []}
        for it in items:
            l = it[3]
            if force:
                sel[force].append(it)
                continue
            cost = {"dve": _C_DVE[0] + _C_DVE[1] * l,
                    "act": _C_ACT[0] + _C_ACT[1] * l,
                    "pool": _C_POOL[0] + _C_POOL[1] * l}
            for e_ in skip:
                cost.pop(e_, None)
            e = min(cost, key=lambda e_: loads[e_] + cost[e_])
            loads[e] += cost[e]
            sel[e].append(it)
        norm_by_chunk.append(sel)

    return dict(segs=segs, nseg=nseg, nsegp=nsegp, chunks=chunks, xcol=xcol,
                csmax=csmax, invp=invp, triv=triv,
                norm_by_chunk=norm_by_chunk)


def _build(plan):
    f32 = mybir.dt.float32
    f16 = mybir.dt.float16
    TRIV = plan["triv"]
    segs = plan["segs"]
    chunks = plan["chunks"]
    xcol = plan["xcol"]
    norm_by_chunk = plan["norm_by_chunk"]
    nseg, nsegp, csmax = plan["nseg"], plan["nsegp"], plan["csmax"]
    nchunk = len(chunks)
    NW = nchunk + 2          # pipeline windows
    NCONST = 2 if TRIV else 4

    nc = bass.Bass()
    xt = nc.declare_dram_parameter("xt", [NU * 128, S2], f32, isOutput=False)
    invpd = nc.declare_dram_parameter("invp", [nsegp], f16, isOutput=False)
    wbd = bbd = None
    if not TRIV:
        wbd = nc.declare_dram_parameter("wbx", [128, nsegp * NL], f32,
                                        isOutput=False)
        bbd = nc.declare_dram_parameter("bbx", [128, nsegp * NL], f32,
                                        isOutput=False)
    epsd = nc.declare_dram_parameter("epsv", [128, 1], f32, isOutput=False)
    yt = nc.declare_dram_parameter("yt", [NU * 128, S2], f16, isOutput=True)

    def chunk_geom(c):
        a, b = chunks[c]
        col0 = segs[a][0]
        cs = sum(segs[k][1] for k in range(a, b))
        return a, b, col0, cs

    def dram3(dram, col0, cs):
        ap = dram[:, :]
        return bass.AP(tensor=ap.tensor, offset=ap.offset + 2 * col0,
                       ap=[[S2, 128], [128 * S2, NU], [1, 2 * cs]])

    from contextlib import ExitStack
    ctx = ExitStack()
    with ctx:
        xb = [ctx.enter_context(nc.sbuf_tensor("xb%d" % i,
                                               [128, NU, 2 * csmax], f32))
              for i in range(NBUF)]
        yb = [ctx.enter_context(nc.sbuf_tensor("yb%d" % i,
                                               [128, NU, 2 * csmax], f16))
              for i in range(YBUF)]
        # s6 lanes per (seg, unit): (cnt, mean_f0, M2_f0, cnt, mean_f1, M2_f1)
        s6 = ctx.enter_context(nc.sbuf_tensor([128, nsegp, NU, 6], f32))
        # per-(seg, unit, parity) stat arrays, lane order (u, par)
        tvar = ctx.enter_context(nc.sbuf_tensor([128, nsegp, NU, 2], f32))
        tstd = ctx.enter_context(nc.sbuf_tensor([128, nsegp, NU, 2], f32))
        trstd = ctx.enter_context(nc.sbuf_tensor([128, nsegp, NU, 2], f32))
        t6 = ctx.enter_context(nc.sbuf_tensor([128, nsegp, NU, 2], f32))
        t7 = ctx.enter_context(nc.sbuf_tensor([128, nsegp, NU, 2], f32))
        Ct = ctx.enter_context(nc.sbuf_tensor([128, nsegp, NU, 2], f32))
        At = ctx.enter_context(nc.sbuf_tensor([128, nsegp, NU, 2], f32))
        invpt = ctx.enter_context(nc.sbuf_tensor([128, nsegp], f16))
        wexpt = bexpt = None
        if not TRIV:
            wexpt = ctx.enter_context(
                nc.sbuf_tensor([128, nsegp, NU, 2], f32))
            bexpt = ctx.enter_context(
                nc.sbuf_tensor([128, nsegp, NU, 2], f32))
        epst = ctx.enter_context(nc.sbuf_tensor([128, 1], f32))
        dva = ctx.enter_context(nc.sbuf_tensor([128, 2], f32))
        dac = ctx.enter_context(nc.sbuf_tensor([128, 2], f32))
        dpl = ctx.enter_context(nc.sbuf_tensor([128, 2], f32))
        LDC = ctx.enter_context(nc.semaphore("LDC"))
        LD = ctx.enter_context(nc.semaphore("LD"))
        BNS = ctx.enter_context(nc.semaphore("BNS"))
        CMB = ctx.enter_context(nc.semaphore("CMB"))
        SQT = ctx.enter_context(nc.semaphore("SQT"))
        COEF = ctx.enter_context(nc.semaphore("COEF"))
        ND = ctx.enter_context(nc.semaphore("ND"))
        NA = ctx.enter_context(nc.semaphore("NA"))
        NP = ctx.enter_context(nc.semaphore("NP"))
        ST = ctx.enter_context(nc.semaphore("ST"))
        block = ctx.enter_context(nc.Block())

        AOP = mybir.AluOpType
        AFT = mybir.ActivationFunctionType

        def stt(eng, out, in0, in1, op1, scalar=0.0, op0=AOP.add):
            return eng.scalar_tensor_tensor(
                out=out, in0=in0, scalar=scalar, in1=in1, op0=op0, op1=op1)

        def emit_norm(eng, c, items):
            """Emit normalize ops for chunk c's items on one engine.
            Items are (u, k, off_rel, len, parity); x/y are accessed
            through stride-2 column APs."""
            insts = []
            for (u, k, o, l, par) in items:
                xin = xb[c % NBUF][:, u, 2 * o + par:2 * (o + l) - 1 + par:2]
                yout = yb[c % YBUF][:, u, 2 * o + par:2 * (o + l) - 1 + par:2]
                Asrc = trstd if TRIV else At
                if eng == "act":
                    i = nc.scalar.activation(
                        out=yout, in_=xin, func=AFT.Identity,
                        scale=Asrc[:, k, u, par:par + 1],
                        bias=Ct[:, k, u, par:par + 1])
                else:
                    e = nc.vector if eng == "dve" else nc.gpsimd
                    i = e.tensor_scalar(
                        out=yout, in0=xin,
                        scalar1=Asrc[:, k, u, par:par + 1],
                        scalar2=Ct[:, k, u, par:par + 1],
                        op0=AOP.mult, op1=AOP.add)
                insts.append(i)
            return insts

        # ---------------- SP: loads + stores --------------------------------
        @block.sync
        def _(sp):
            for h in (ND, NA, NP):
                sp.sem_clear(h)

            def bcast(dram, n):
                ap = dram[:]
                return bass.AP(tensor=ap.tensor, offset=ap.offset,
                               ap=[[0, 128], [1, n]])
            a, b, col0, cs = chunk_geom(0)
            sp.dma_start(out=xb[0][:, :, 0:2 * cs],
                         in_=dram3(xt, col0, cs)).then_inc(LD, 16)
            sp.dma_start(out=invpt[:, :], in_=bcast(invpd, nsegp)
                         ).then_inc(LDC, 16)
            if not TRIV:
                sp.dma_start(out=wexpt[:, :, :, :], in_=wbd[:, :]
                             ).then_inc(LDC, 16)
                sp.dma_start(out=bexpt[:, :, :, :], in_=bbd[:, :]
                             ).then_inc(LDC, 16)
            sp.dma_start(out=epst[:, :], in_=epsd[:, :]).then_inc(LDC, 16)
            for c in range(1, min(NBUF, nchunk)):
                a, b, col0, cs = chunk_geom(c)
                sp.dma_start(out=xb[c % NBUF][:, :, 0:2 * cs],
                             in_=dram3(xt, col0, cs)).then_inc(LD, 16)
            for c in range(nchunk):
                sp.wait_ge(ND, c + 1)
                sp.wait_ge(NA, c + 1)
                sp.wait_ge(NP, c + 1)
                a, b, col0, cs = chunk_geom(c)
                sp.dma_start(out=dram3(yt, col0, cs),
                             in_=yb[c % YBUF][:, :, 0:2 * cs]
                             ).then_inc(ST, 16)
                if c + NBUF < nchunk:
                    a2, b2, col02, cs2 = chunk_geom(c + NBUF)
                    sp.dma_start(out=xb[(c + NBUF) % NBUF][:, :, 0:2 * cs2],
                                 in_=dram3(xt, col02, cs2)).then_inc(LD, 16)

        # ---------------- DVE: bn_stats, recip+coefs, tiny normalize --------
        @block.vector
        def _(ve):
            for h in (LD, LDC, SQT, COEF, ST):
                ve.sem_clear(h)
            rsp = int(os.environ.get("KERNEL_RSP", "2"))
            for w in range(NW):
                c_bn, c_rc, c_nm = w, w - 1, w - 2
                if c_bn < nchunk:
                    a, b, col0, cs = chunk_geom(c_bn)
                    ve.wait_ge(LD, 16 * (c_bn + 1))
                    last = None
                    for k in range(a, b):
                        s0, ln = segs[k]
                        o = s0 - col0
                        l1 = min(ln, ICL)
                        for u in range(NU):
                            last = nc.vector.bn_stats(
                                out=s6[:, k, u, :],
                                in_=xb[c_bn % NBUF][:, u, 2 * o:2 * (o + l1)])
                            if ln > ICL:
                                last = nc.vector.bn_stats(
                                    out=s6[:, xcol[k], u, :],
                                    in_=xb[c_bn % NBUF][
                                        :, u, 2 * (o + l1):2 * (o + ln)])
                    last.then_inc(BNS, 1)
                nitems = norm_by_chunk[c_nm]["dve"] \
                    if 0 <= c_nm < nchunk else []
                ni = 0

                def filler(n, c_nm=c_nm, nitems=nitems):
                    nonlocal ni
                    out = []
                    take = nitems[ni:ni + n]
                    if take:
                        out = emit_norm("dve", c_nm, take)
                        ni += len(take)
                    for _ in range(n - len(take)):
                        out.append(nc.vector.memset(dva[:, 0:2], 0.0))
                    return out

                if 0 <= c_nm < nchunk:
                    ve.wait_ge(COEF, c_nm + 1)
                    if c_nm >= YBUF - 1:
                        ve.wait_ge(ST, 16 * (c_nm - YBUF + 2))
                if 0 <= c_rc < nchunk:
                    if c_rc == 0:
                        ve.wait_ge(LDC, 16 * NCONST)
                    a, b, col0, cs = chunk_geom(c_rc)
                    ve.wait_ge(SQT, c_rc + 1)
                    sl = slice(a, b)
                    filler(rsp)
                    nc.vector.reciprocal(
                        out=trstd[:, sl, :, :], in_=tstd[:, sl, :, :])
                    filler(2)
                    # mean lanes of s6: [..., 1] and [..., 4] -> (u, par)
                    mean_sl = s6[:, sl, :, 1::3]
                    if TRIV:
                        stt(nc.vector, t6[:, sl, :, :], mean_sl,
                            trstd[:, sl, :, :], AOP.mult)
                        filler(2)
                        nc.vector.tensor_scalar_mul(
                            out=Ct[:, sl, :, :], in0=t6[:, sl, :, :],
                            scalar1=-1.0).then_inc(COEF, 1)
                    else:
                        stt(nc.vector, At[:, sl, :, :], trstd[:, sl, :, :],
                            wexpt[:, sl, :, :], AOP.mult)
                        filler(2)
                        stt(nc.vector, t6[:, sl, :, :], mean_sl,
                            At[:, sl, :, :], AOP.mult)
                        filler(2)
                        stt(nc.vector, Ct[:, sl, :, :], bexpt[:, sl, :, :],
                            t6[:, sl, :, :], AOP.subtract).then_inc(COEF, 1)
                if 0 <= c_nm < nchunk:
                    insts = filler(max(0, len(nitems) - ni))
                    if insts:
                        insts[-1].then_inc(ND, 1)
                    else:
                        nc.vector.memset(dva[:, 0:1], 0.0).then_inc(ND, 1)

        # ---------------- ACT: sqrt + long normalize ------------------------
        @block.scalar
        def _(ac):
            for h in (CMB, COEF, ST, LDC):
                ac.sem_clear(h)
            ac.wait_ge(LDC, 16 * NCONST)
            adl = int(os.environ.get("KERNEL_ACT_DELAY", "1"))
            for w in range(NW):
                c_sq, c_nm = w - 1, w - 2
                if 0 <= c_sq < nchunk:
                    a, b, col0, cs = chunk_geom(c_sq)
                    ac.wait_ge(CMB, c_sq + 1)
                    for _ in range(adl):
                        nc.scalar.activation(
                            out=dac[:, 0:1], in_=dac[:, 0:1], func=AFT.Copy)
                    nc.scalar.activation(
                        out=tstd[:, a:b, :, :], in_=tvar[:, a:b, :, :],
                        func=AFT.Sqrt, bias=epst[:, 0:1], scale=1.0
                    ).then_inc(SQT, 1)
                if 0 <= c_nm < nchunk:
                    ac.wait_ge(COEF, c_nm + 1)
                    if c_nm >= YBUF - 1:
                        ac.wait_ge(ST, 16 * (c_nm - YBUF + 2))
                    insts = emit_norm("act", c_nm, norm_by_chunk[c_nm]["act"])
                    if insts:
                        insts[-1].then_inc(NA, 1)
                    else:
                        nc.scalar.activation(
                            out=dac[:, 0:1], in_=dac[:, 0:1],
                            func=AFT.Copy).then_inc(NA, 1)

        # ---------------- Pool: mid normalize, var op -----------------------
        @block.gpsimd
        def _(g):
            for h in (BNS, COEF, ST, LDC):
                g.sem_clear(h)

            for w in range(NW):
                c_cb, c_nm = w, w - 2
                if 0 <= c_nm < nchunk:
                    g.wait_ge(COEF, c_nm + 1)
                    if c_nm >= YBUF - 1:
                        g.wait_ge(ST, 16 * (c_nm - YBUF + 2))
                    insts = emit_norm("pool", c_nm,
                                      norm_by_chunk[c_nm]["pool"])
                    if insts:
                        insts[-1].then_inc(NP, 1)
                    else:
                        nc.gpsimd.memset(dpl[:, 0:1], 0.0).then_inc(NP, 1)
                if c_cb < nchunk:
                    a, b, col0, cs = chunk_geom(c_cb)
                    g.wait_ge(BNS, c_cb + 1)
                    if c_cb == 0:
                        g.wait_ge(LDC, 16 * NCONST)
                    # combine the two pieces of long segments (immediate
                    # coefficients; writes back into piece-1 mean/M2 lanes)
                    for k in range(a, b):
                        s0, ln = segs[k]
                        if ln <= ICL:
                            continue
                        l1 = ICL
                        l2 = ln - ICL
                        kx = xcol[k]
                        for u in range(NU):
                            m1 = s6[:, k, u, 1::3]     # [128, 2]
                            q1 = s6[:, k, u, 2::3]
                            m2 = s6[:, kx, u, 1::3]
                            q2 = s6[:, kx, u, 2::3]
                            mm = t6[:, k, u, :]        # scratch [128, 2]
                            qq = t7[:, k, u, :]
                            # mm = m1*(l1/ln) + m2*(l2/ln)
                            nc.gpsimd.tensor_scalar_mul(
                                out=mm, in0=m1, scalar1=float(l1) / ln)
                            stt(nc.gpsimd, mm, m2, mm, AOP.add,
                                scalar=float(l2) / ln, op0=AOP.mult)
                            # qq = q1 + q2 + l1*m1^2 + l2*m2^2
                            stt(nc.gpsimd, qq, q1, q2, AOP.add)
                            nc.gpsimd.tensor_scalar(
                                out=At[:, k, u, :], in0=m1,
                                scalar1=float(l1), scalar2=0.0,
                                op0=AOP.mult, op1=AOP.add)
                            stt(nc.gpsimd, At[:, k, u, :], m1,
                                At[:, k, u, :], AOP.mult)
                            stt(nc.gpsimd, qq, At[:, k, u, :], qq, AOP.add)
                            nc.gpsimd.tensor_scalar(
                                out=At[:, k, u, :], in0=m2,
                                scalar1=float(l2), scalar2=0.0,
                                op0=AOP.mult, op1=AOP.add)
                            stt(nc.gpsimd, At[:, k, u, :], m2,
                                At[:, k, u, :], AOP.mult)
                            stt(nc.gpsimd, qq, At[:, k, u, :], qq, AOP.add)
                            # M2_comb = qq - ln*mm^2; write back into lanes
                            nc.gpsimd.tensor_scalar(
                                out=At[:, k, u, :], in0=mm,
                                scalar1=float(ln), scalar2=0.0,
                                op0=AOP.mult, op1=AOP.add)
                            stt(nc.gpsimd, At[:, k, u, :], mm,
                                At[:, k, u, :], AOP.mult)
                            stt(nc.gpsimd, q1, qq, At[:, k, u, :],
                                AOP.subtract)
                            nc.gpsimd.tensor_scalar_mul(
                                out=m1, in0=mm, scalar1=1.0)
                    # var = M2 lane * (1/len); lanes [...,2] and [...,5]
                    iv = invpt[:, a:b]
                    iv_b = bass.AP(
                        tensor=iv.tensor, offset=iv.offset,
                        ap=[[int(s_), int(n_)] for s_, n_ in iv.ap[:-1]] +
                           [[int(iv.ap[-1][0]), int(iv.ap[-1][1])],
                            [0, NU], [0, 2]])
                    stt(nc.gpsimd, tvar[:, a:b, :, :], s6[:, a:b, :, 2::3],
                        iv_b, AOP.mult).then_inc(CMB, 1)

    return nc


def kernel(x, affine_weight, affine_bias, change_points):
    x = np.asarray(x, dtype=np.float32)
    w = np.asarray(affine_weight, dtype=np.float32)
    bb = np.asarray(affine_bias, dtype=np.float32)
    cp = np.asarray(change_points)

    triv = bool(np.all(w == 1.0) and np.all(bb == 0.0))
    plan = _plan(cp, triv)
    sig = (triv,) + tuple(s for s, _ in plan["segs"])
    if sig not in _cache:
        _cache[sig] = _build(plan)
    nc = _cache[sig]

    nsegp = plan["nsegp"]
    epsv = np.full((128, 1), EPS, np.float32)
    wbx = bbx = None
    if not triv:
        # lane order (k, u, par): feature = par*128 + p
        wbx = np.zeros((128, nsegp * NL), np.float32)
        bbx = np.zeros((128, nsegp * NL), np.float32)
        for u in range(NU):
            for par in range(2):
                lane = u * 2 + par
                wbx[:, lane::NL] = w[par * 128:(par + 1) * 128][:, None]
                bbx[:, lane::NL] = bb[par * 128:(par + 1) * 128][:, None]

    in_maps = []
    for i in range(NCORES):
        xc = x[i * BPC:(i + 1) * BPC]              # [NU, S, F]
        # row (u*128+p) = interleave(x[u, :, p], x[u, :, p+128])
        xi = np.empty((NU, 128, S2), np.float32)
        xv = xc.transpose(0, 2, 1)                 # [NU, F, S]
        xi[:, :, 0::2] = xv[:, 0:128, :]
        xi[:, :, 1::2] = xv[:, 128:256, :]
        m = {"xt": xi.reshape(NU * 128, S2), "invp": plan["invp"],
             "epsv": epsv}
        if not triv:
            m["wbx"] = wbx
            m["bbx"] = bbx
        in_maps.append(m)

    res = run_bass_kernel_spmd(nc, in_maps, core_ids=list(range(NCORES)),
                               trace=False)

    y = np.empty((B, S, F), np.float32)
    for i in range(NCORES):
        yi = np.asarray(res.results[i]["yt"]).astype(np.float32)
        yi = yi.reshape(NU, 128, S2)
        yv = np.empty((NU, F, S), np.float32)
        yv[:, 0:128, :] = yi[:, :, 0::2]
        yv[:, 128:256, :] = yi[:, :, 1::2]
        y[i * BPC:(i + 1) * BPC] = yv.transpose(0, 2, 1)
    return y
